# revision 1
# baseline (speedup 1.0000x reference)
"""Trainium2 Bass kernel for nn_F_VAE_can_7902739824969.

Reference, per batch row b with domain d = dom[b]:
    out[b] = F_d @ eps[b] + concat(bias_shared, bias_nonshared[d])
with F_d = (I - L_d)^{-1} S_d, L_d strictly-lower only in the last K=64 rows,
S_d diagonal.  Hence F_d = [[I, 0], [F21_d, F22_d]]: only the bottom K rows
(F_bot, [D, K, N]) carry information:
    out[b, :N-K] = eps[b, :N-K] + bias_shared
    out[b, N-K:] = F_bot[d] @ eps[b] + bias_nonshared[d]

Host (inside kernel()): solve the D unit-triangular systems for F_bot, sort
batch rows by domain (sharding permutation), give each of 8 cores 128 sorted
rows plus only the few domain blocks of F^T that shard touches.

Device (raw bacc, no Tile, ~30 instructions): the two HWDGE queues (sync,
scalar) each carry one merged big-packet transfer holding [epsT|F^T] chunk
pairs; PE runs a 4-chunk fp32 accumulation chain into PSUM per segment bank;
DVE adds the shared bias to the top 448 columns and does the masked segment
select + nonshared-bias add for the bottom 64; the two output DMAs ride the
two queues in parallel.  No explicit teardown: the NEFF epilogue drains the
queues and the next execution's Bass prologue clears the semaphore range.
"""

import numpy as np

B = 1024
N = 512
K = 64
D = 16
P = 128
NC = 8
RPC = B // NC          # rows per core
NTOP = N - K           # 448
NCHUNK = N // P        # 4 contraction chunks

# "float32": exact, but fp32 matmuls run 4 cycles/row (2 passes).
# "float32r": TF32-like (~1.5e-4 rel), 1 cycle/row when free dim >= 256.
MM_DTYPE = "float32"
MIN_NSEG = {"float32": 1, "float32r": 4}

_PROG_CACHE: dict = {}


def _build_fbot(L_emb, S_emb):
    """F_bot [D, K, N] (float32): bottom K rows of (I - L_d)^{-1} S_d."""
    L_emb = np.asarray(L_emb, np.float64)
    S_emb = np.asarray(S_emb, np.float64)
    off = np.zeros(K, dtype=np.int64)
    for r in range(1, K):
        off[r] = off[r - 1] + (NTOP + r - 1)
    L21 = np.zeros((D, K, NTOP))
    L22 = np.zeros((D, K, K))
    for r in range(K):
        L21[1:, r, :] = L_emb[1:, off[r] : off[r] + NTOP]
        if r > 0:
            L22[1:, r, :r] = L_emb[1:, off[r] + NTOP : off[r] + NTOP + r]
    s = np.ones((D, K))
    s[1:] = S_emb[1:]
    rhs = np.concatenate([L21, s[:, :, None] * np.eye(K)[None]], axis=2)  # [D,K,N]
    X = np.zeros_like(rhs)
    for r in range(K):
        X[:, r, :] = rhs[:, r, :] + np.einsum(
            "dj,djn->dn", L22[:, r, :r], X[:, :r, :]
        )
    return X.astype(np.float32)


def _seg_layout(nseg):
    """Split nseg segments into PSUM banks of <= 8 (K*8 fp32 = one 2KB bank)."""
    banks = []
    s0 = 0
    while s0 < nseg:
        nb = min(8, nseg - s0)
        banks.append((s0, nb))
        s0 += nb
    return banks


def _build_program(nseg, mm_dt_name=MM_DTYPE):
    import concourse.bacc as bacc
    import concourse.mybir as mybir

    f32 = mybir.dt.float32
    mmdt = getattr(mybir.dt, mm_dt_name)
    banks = _seg_layout(nseg)  # PSUM banks of <= 8 segments each
    fta_cols = K * nseg

    mmw = P + fta_cols  # per-chunk block: [epsT chunk | fta chunk]
    NG = NCHUNK // 2    # chunk pairs, one DMA each (bigger packets)

    auxw = 2 * NTOP + nseg + K  # [eps_top | bias_top | masks | bbot] per row

    nc = bacc.Bacc()
    mm_in = nc.declare_dram_parameter("mm", [NG, P, 2 * mmw], mmdt, isOutput=False)
    aux_in = nc.declare_dram_parameter("aux", [RPC, auxw], f32, isOutput=False)
    out_ext = nc.declare_dram_parameter("out", [RPC, N], f32, isOutput=True)

    mm_sb = nc.alloc_sbuf_tensor("mm_sb", [P, NCHUNK, mmw], mmdt).ap()
    aux_sb = nc.alloc_sbuf_tensor("aux_sb", [P, auxw], f32).ap()
    # +1 trailing slice per first bank holds bbot so the reduce emits
    # (masked sum + nonshared bias) in one pass
    tmp_sb = [
        nc.alloc_sbuf_tensor(f"tmp_sb{bi}", [P, K, nb + (bi == 0)], f32).ap()
        for bi, (s0, nb) in enumerate(banks)
    ]
    red_sb = [
        nc.alloc_sbuf_tensor(f"red_sb{bi}", [P, K], f32).ap()
        for bi in range(len(banks))
    ]
    out_sb = nc.alloc_sbuf_tensor("out_sb", [P, N], f32).ap()
    pz = [
        nc.alloc_psum_tensor(f"pz{bi}", [P, K, nb], f32).ap()
        for bi, (s0, nb) in enumerate(banks)
    ]
    eps_top_sb = aux_sb[:, :NTOP]
    ptop_sb = aux_sb[:, NTOP : 2 * NTOP]
    masks_sb = aux_sb[:, 2 * NTOP : 2 * NTOP + nseg]
    bbot_sb = aux_sb[:, 2 * NTOP + nseg :]

    s_pair = [nc.alloc_semaphore(f"s_pr{g}") for g in range(NG)]
    s_aux = nc.alloc_semaphore("s_aux")
    s_pe = nc.alloc_semaphore("s_pe")
    s_dve = nc.alloc_semaphore("s_dve")
    s_out = nc.alloc_semaphore("s_out")

    with nc.Block() as block:

        @block.tensor
        def _(te):
            mm = None
            for c in range(NCHUNK):
                if c % 2 == 0:
                    te.wait_ge(s_pair[c // 2], 16)
                for bi, (s0, nb) in enumerate(banks):
                    cols = slice(P + K * s0, P + K * (s0 + nb))
                    mm = te.matmul(
                        pz[bi],
                        lhsT=mm_sb[:, c, :P],
                        rhs=mm_sb[:, c, cols],
                        start=(c == 0),
                        stop=(c == NCHUNK - 1),
                    )
            mm.then_inc(s_pe, 1)

        # No explicit teardown: the NEFF epilogue drains the DMA queues (the
        # runtime cannot return before output DMAs land), and the Bass
        # prologue of the next execution dma_reset+sem_clears the whole
        # kernel semaphore range before any use.


        @block.scalar
        def _(sc):
            sc.dma_start(
                mm_sb[:, 2:4, :].rearrange("p c w -> p (c w)"), mm_in[1]
            ).then_inc(s_pair[1], 16)
            sc.wait_ge(s_dve, 2)
            sc.dma_start(out_ext[:, NTOP:], out_sb[:, NTOP:]).then_inc(s_out, 16)

        @block.sync
        def _(sy):
            sy.dma_start(
                mm_sb[:, 0:2, :].rearrange("p c w -> p (c w)"), mm_in[0]
            ).then_inc(s_pair[0], 16)
            sy.dma_start(aux_sb, aux_in[:]).then_inc(s_aux, 16)
            sy.wait_ge(s_dve, 1)
            sy.dma_start(out_ext[:, :NTOP], out_sb[:, :NTOP]).then_inc(s_out, 16)

        @block.vector
        def _(ve):
            ve.wait_ge(s_aux, 16)
            # stage bbot into bank 0's trailing reduce slice (off critical path)
            nb0 = banks[0][1]
            ve.tensor_copy(tmp_sb[0][:, :, nb0], bbot_sb)
            ve.tensor_tensor(
                out_sb[:, :NTOP], eps_top_sb, ptop_sb, mybir.AluOpType.add
            ).then_inc(s_dve, 1)
            ve.wait_ge(s_pe, 1)
            nbanks = len(banks)
            for bi, (s0, nb) in enumerate(banks):
                ve.tensor_tensor(
                    tmp_sb[bi][:, :, :nb],
                    pz[bi],
                    masks_sb[:, None, s0 : s0 + nb].to_broadcast([P, K, nb]),
                    mybir.AluOpType.mult,
                )
                ve.drain()  # same-engine RAW through SBUF needs a drain
                out_ap = out_sb[:, NTOP:] if bi == 0 else red_sb[bi]
                ve.tensor_reduce(
                    out_ap,
                    tmp_sb[bi][:, :, : nb + (bi == 0)],
                    mybir.AxisListType.X,
                    mybir.AluOpType.add,
                )
                if bi > 0 or bi == nbanks - 1:
                    ve.drain()
                if bi > 0:
                    ve.tensor_tensor(
                        out_sb[:, NTOP:], out_sb[:, NTOP:], red_sb[bi],
                        mybir.AluOpType.add,
                    )
                    ve.drain()
            ve.sem_inc(s_dve, 1)

    nc.compile()
    return nc


def _prepare(epsilon, d, L_emb, S_emb, bias_nonshared, bias_shared,
             mm_dt_name=MM_DTYPE):
    """Host-side sharding. Returns (nseg, in_maps, perm)."""
    eps = np.ascontiguousarray(np.asarray(epsilon, np.float32))
    dv = np.asarray(d).astype(np.int64).reshape(B)
    bias_ns = np.asarray(bias_nonshared, np.float32)
    bias_sh = np.asarray(bias_shared, np.float32).reshape(1, NTOP)

    fbot = _build_fbot(L_emb, S_emb)                     # [D, K, N]
    ft = np.ascontiguousarray(fbot.transpose(0, 2, 1))   # [D, N, K]

    perm = np.argsort(dv, kind="stable")
    ds_sorted = dv[perm]
    eps_sorted = eps[perm]

    shard_segs = []
    for c in range(NC):
        rows = ds_sorted[c * RPC : (c + 1) * RPC]
        segs = []
        for dd in rows:
            if not segs or segs[-1] != dd:
                segs.append(int(dd))
        shard_segs.append(segs)
    nseg = max(len(s) for s in shard_segs)
    nseg = max(nseg, MIN_NSEG.get(mm_dt_name, 1))

    fta_cols = K * nseg
    in_maps = []
    for c in range(NC):
        segs = shard_segs[c]
        rows = ds_sorted[c * RPC : (c + 1) * RPC]
        eps_c = np.ascontiguousarray(eps_sorted[c * RPC : (c + 1) * RPC])
        fta = np.zeros((N, fta_cols), np.float32)
        masks = np.zeros((RPC, nseg), np.float32)
        for s, dd in enumerate(segs):
            # bank-local interleave: col = K*s0 + k*nb + (s - s0)
            for s0, nb in _seg_layout(nseg):
                if s0 <= s < s0 + nb:
                    cols = K * s0 + np.arange(K) * nb + (s - s0)
                    break
            fta[:, cols] = ft[dd]
            masks[:, s] = (rows == dd).astype(np.float32)
        # everything the DVE needs, one row-contiguous block per batch row:
        # [eps_top | bias_top | masks | bbot]
        aux = np.concatenate(
            [
                eps_c[:, :NTOP],
                np.broadcast_to(bias_sh, (RPC, NTOP)),
                masks,
                bias_ns[rows],
            ],
            axis=1,
        ).astype(np.float32)
        # merged matmul input, one block per chunk PAIR: partition p holds
        # [epsT c | fta c | epsT c+1 | fta c+1] contiguous -> big DMA packets
        mmw = P + fta_cols
        mm = np.empty((NCHUNK // 2, P, 2 * mmw), np.float32)
        for ci in range(NCHUNK):
            g, h = divmod(ci, 2)
            mm[g, :, h * mmw : h * mmw + P] = eps_c[:, ci * P : (ci + 1) * P].T
            mm[g, :, h * mmw + P : (h + 1) * mmw] = fta[ci * P : (ci + 1) * P, :]
        in_maps.append({"mm": mm, "aux": np.ascontiguousarray(aux)})
    return nseg, in_maps, perm


def _finish(results, perm):
    out_sorted = np.concatenate([results[c]["out"] for c in range(NC)], axis=0)
    out = np.empty((B, N), np.float32)
    out[perm] = out_sorted
    return out


def get_program(nseg, mm_dt_name=MM_DTYPE):
    key = (nseg, mm_dt_name)
    prog = _PROG_CACHE.get(key)
    if prog is None:
        prog = _build_program(nseg, mm_dt_name)
        _PROG_CACHE[key] = prog
    return prog


def kernel(epsilon, d, L_emb, S_emb, bias_nonshared, bias_shared):
    from concourse.bass_utils import run_bass_kernel_spmd

    nseg, in_maps, perm = _prepare(
        epsilon, d, L_emb, S_emb, bias_nonshared, bias_shared
    )
    prog = get_program(nseg)
    res = run_bass_kernel_spmd(prog, in_maps, list(range(NC))).results
    return _finish(res, perm)



# revision 3
# speedup vs baseline: 1.2237x; 1.2237x over previous
"""Trainium2 Bass kernel for nn_F_VAE_can_7902739824969.

Reference, per batch row b with domain d = dom[b]:
    out[b] = F_d @ eps[b] + concat(bias_shared, bias_nonshared[d])
with F_d = (I - L_d)^{-1} S_d, L_d strictly-lower only in the last K=64 rows,
S_d diagonal.  Hence F_d = [[I, 0], [F21_d, F22_d]]: only the bottom K rows
(F_bot, [D, K, N]) carry information:
    out[b, :N-K] = eps[b, :N-K] + bias_shared
    out[b, N-K:] = F_bot[d] @ eps[b] + bias_nonshared[d]

Host (inside kernel()): solve the D unit-triangular systems for F_bot, sort
batch rows by domain, give each of 8 cores 128 sorted rows plus the few
domain blocks of F^T that shard touches.  Everything ships as bf16 (the
correctness gate is rel 2e-2; bf16 keeps us ~5e-3).

Device, per core (raw bacc):
  PE does ALL the math into two PSUM banks:
    p_top [128,448] = bias_top (rank-1 ones x bias matmul)
                      + per-chunk identity matmuls (epsT_c^T @ I = eps chunk,
                        i.e. the PE doubles as the eps transpose engine)
    p_bot [128,K*nseg] = bbot (rank-1) + per-chunk epsT_c^T @ F^T chunks
  DVE copies p_top -> out rows 0:448 (early, overlaps the F matmuls),
  then mask-multiplies p_bot per segment and reduces into out rows 448:512.
  Two output DMAs (top early / bot late) overlap the PE tail.
  Cheap bf16 dummy matmuls keep the PE clock-gate (HAM) open: warm PE both
  speeds the real matmuls and speeds the Tensor section of the runtime's
  end-of-NEFF semaphore-clear ladder (~115 vs ~140 ns per clear).
"""

import numpy as np
import ml_dtypes

B = 1024
N = 512
K = 64
D = 16
P = 128
NC = 8
RPC = B // NC          # rows per core
NTOP = N - K           # 448
NCHUNK = N // P        # 4 contraction chunks

BF16 = ml_dtypes.bfloat16

# PE keep-warm dummy matmuls (each ~100ns bf16): W_START run while the input
# DMAs are in flight, W_TAIL after the real matmuls while DVE + output DMAs
# finish.  Both only burn otherwise-idle PE time.
W_START = 16
W_TAIL = 14

_PROG_CACHE: dict = {}


def _build_fbot(L_emb, S_emb):
    """F_bot [D, K, N] (float32): bottom K rows of (I - L_d)^{-1} S_d."""
    L_emb = np.asarray(L_emb, np.float64)
    S_emb = np.asarray(S_emb, np.float64)
    off = np.zeros(K, dtype=np.int64)
    for r in range(1, K):
        off[r] = off[r - 1] + (NTOP + r - 1)
    L21 = np.zeros((D, K, NTOP))
    L22 = np.zeros((D, K, K))
    for r in range(K):
        L21[1:, r, :] = L_emb[1:, off[r] : off[r] + NTOP]
        if r > 0:
            L22[1:, r, :r] = L_emb[1:, off[r] + NTOP : off[r] + NTOP + r]
    s = np.ones((D, K))
    s[1:] = S_emb[1:]
    rhs = np.concatenate([L21, s[:, :, None] * np.eye(K)[None]], axis=2)  # [D,K,N]
    X = np.zeros_like(rhs)
    for r in range(K):
        X[:, r, :] = rhs[:, r, :] + np.einsum(
            "dj,djn->dn", L22[:, r, :r], X[:, :r, :]
        )
    return X.astype(np.float32)


def _build_program(nseg):
    import concourse.bacc as bacc
    import concourse.mybir as mybir

    f32 = mybir.dt.float32
    bf16 = mybir.dt.bfloat16

    aw = NCHUNK * P + P          # epsT chunks | identity
    bw = NCHUNK * nseg * K       # F^T chunks, (k*nseg+s)-interleaved
    rw = NTOP + nseg * K + P     # bias_top | bbot_flat | ones

    nc = bacc.Bacc()
    a_in = nc.declare_dram_parameter("a", [P, aw], bf16, isOutput=False)
    b_in = nc.declare_dram_parameter("b", [P, bw], bf16, isOutput=False)
    r_in = nc.declare_dram_parameter("r", [2, rw], bf16, isOutput=False)
    m_in = nc.declare_dram_parameter("m", [RPC, nseg], f32, isOutput=False)
    o_ext = nc.declare_dram_parameter("o", [RPC, N], bf16, isOutput=True)

    a_sb = nc.alloc_sbuf_tensor("a_sb", [P, aw], bf16).ap()
    b_sb = nc.alloc_sbuf_tensor("b_sb", [P, bw], bf16).ap()
    r_sb = nc.alloc_sbuf_tensor("r_sb", [2, rw], bf16).ap()
    m_sb = nc.alloc_sbuf_tensor("m_sb", [P, nseg], f32).ap()
    junk = nc.alloc_sbuf_tensor("junk", [P, 32], bf16).ap()
    tmp_sb = nc.alloc_sbuf_tensor("tmp_sb", [P, K, nseg], f32).ap()
    out_sb = nc.alloc_sbuf_tensor("out_sb", [P, N], bf16).ap()

    p_top = nc.alloc_psum_tensor("p_top", [P, NTOP], f32).ap()
    p_bot = nc.alloc_psum_tensor("p_bot", [P, K, nseg], f32).ap()
    p_scr = nc.alloc_psum_tensor("p_scr", [P, 16], f32).ap()

    eye = a_sb[:, NCHUNK * P :]
    ones = r_sb[:, NTOP + nseg * K :]
    bias_top = r_sb[:, :NTOP]
    bbot = r_sb[:, NTOP : NTOP + nseg * K]

    s_junk = nc.alloc_semaphore("s_junk")
    s_a = nc.alloc_semaphore("s_a")
    s_b = nc.alloc_semaphore("s_b")
    s_r = nc.alloc_semaphore("s_r")
    s_m = nc.alloc_semaphore("s_m")
    s_pt = nc.alloc_semaphore("s_pt")
    s_pe = nc.alloc_semaphore("s_pe")
    s_top = nc.alloc_semaphore("s_top")
    s_bot = nc.alloc_semaphore("s_bot")
    s_out = nc.alloc_semaphore("s_out")

    with nc.Block() as block:

        @block.gpsimd
        def _(gp):
            gp.memset(junk, 0).then_inc(s_junk, 1)

        @block.tensor
        def _(te):
            te.wait_ge(s_junk, 1)
            for _ in range(W_START):
                te.matmul(p_scr[:16, :16], lhsT=junk[:, :16], rhs=junk[:, 16:],
                          start=True, stop=True)
            # rank-1 bias fills: p_top = 1 (x) bias_top, p_bot = 1 (x) bbot
            te.wait_ge(s_r, 16)
            te.matmul(p_top, lhsT=ones, rhs=bias_top, start=True, stop=False)
            te.matmul(
                p_bot.rearrange("p k s -> p (k s)"),
                lhsT=ones, rhs=bbot, start=True, stop=False,
            )
            # eps transpose: p_top[:, 128c:...] += epsT_c^T @ I
            te.wait_ge(s_a, 16)
            mm = None
            for c in range(NCHUNK):
                w = P if c < NCHUNK - 1 else P - K
                mm = te.matmul(
                    p_top[:, c * P : c * P + w],
                    lhsT=a_sb[:, c * P : (c + 1) * P],
                    rhs=eye[:, :w],
                    start=False, stop=True,
                )
            mm.then_inc(s_pt, 1)
            # bottom: p_bot += epsT_c^T @ F^T_c
            te.wait_ge(s_b, 16)
            mm = None
            for c in range(NCHUNK):
                mm = te.matmul(
                    p_bot.rearrange("p k s -> p (k s)"),
                    lhsT=a_sb[:, c * P : (c + 1) * P],
                    rhs=b_sb[:, c * nseg * K : (c + 1) * nseg * K],
                    start=False, stop=(c == NCHUNK - 1),
                )
            mm.then_inc(s_pe, 1)
            for _ in range(W_TAIL):
                te.matmul(p_scr[:16, :16], lhsT=junk[:, :16], rhs=junk[:, 16:],
                          start=True, stop=True)

        @block.sync
        def _(sy):
            sy.dma_start(a_sb, a_in[:]).then_inc(s_a, 16)
            sy.dma_start(m_sb, m_in[:]).then_inc(s_m, 16)
            sy.wait_ge(s_top, 1)
            sy.dma_start(o_ext[:, :NTOP], out_sb[:, :NTOP]).then_inc(s_out, 16)

        @block.scalar
        def _(sc):
            sc.dma_start(r_sb, r_in[:]).then_inc(s_r, 16)
            sc.dma_start(b_sb, b_in[:]).then_inc(s_b, 16)
            sc.wait_ge(s_bot, 1)
            sc.dma_start(o_ext[:, NTOP:], out_sb[:, NTOP:]).then_inc(s_out, 16)

        @block.vector
        def _(ve):
            ve.wait_ge(s_pt, 1)
            ve.tensor_copy(out_sb[:, :NTOP], p_top).then_inc(s_top, 1)
            ve.wait_ge(s_pe, 1)
            ve.wait_ge(s_m, 16)
            ve.tensor_tensor(
                tmp_sb,
                p_bot,
                m_sb[:, None, :].to_broadcast([P, K, nseg]),
                mybir.AluOpType.mult,
            )
            ve.drain()  # same-engine RAW through SBUF
            with nc.allow_low_precision("bf16 output; gate is rel 2e-2"):
                ve.tensor_reduce(
                    out_sb[:, NTOP:], tmp_sb, mybir.AxisListType.X,
                    mybir.AluOpType.add,
                )
            ve.drain()
            ve.sem_inc(s_bot, 1)

    nc.compile()
    return nc


def _prepare(epsilon, d, L_emb, S_emb, bias_nonshared, bias_shared):
    """Host-side sharding. Returns (nseg, in_maps, perm)."""
    eps = np.ascontiguousarray(np.asarray(epsilon, np.float32))
    dv = np.asarray(d).astype(np.int64).reshape(B)
    bias_ns = np.asarray(bias_nonshared, np.float32)
    bias_sh = np.asarray(bias_shared, np.float32).reshape(NTOP)

    fbot = _build_fbot(L_emb, S_emb)                     # [D, K, N]

    perm = np.argsort(dv, kind="stable")
    ds_sorted = dv[perm]
    eps_sorted = eps[perm]

    shard_segs = []
    for c in range(NC):
        rows = ds_sorted[c * RPC : (c + 1) * RPC]
        segs = []
        for dd in rows:
            if not segs or segs[-1] != dd:
                segs.append(int(dd))
        shard_segs.append(segs)
    nseg = max(len(s) for s in shard_segs)

    rw = NTOP + nseg * K + P
    in_maps = []
    for c in range(NC):
        segs = shard_segs[c]
        rows = ds_sorted[c * RPC : (c + 1) * RPC]
        eps_c = eps_sorted[c * RPC : (c + 1) * RPC]          # [128, 512]

        # a: epsT chunks | identity.  a[p, cc*128 + r] = eps[r, cc*128 + p]
        a = np.empty((P, NCHUNK * P + P), np.float32)
        est = eps_c.T.reshape(NCHUNK, P, RPC)                # [cc, p, r]
        a[:, : NCHUNK * P] = est.transpose(1, 0, 2).reshape(P, NCHUNK * P)
        a[:, NCHUNK * P :] = np.eye(P, dtype=np.float32)

        # b: F^T chunks, col (cc, k, s) -> fbot[dom_s, k, cc*128+p]
        b = np.zeros((P, NCHUNK, K, nseg), np.float32)
        for s, dd in enumerate(segs):
            b[:, :, :, s] = fbot[dd].T.reshape(NCHUNK, P, K).transpose(1, 0, 2)
        b = b.reshape(P, NCHUNK * nseg * K)

        # r: bias_top | bbot_flat | ones (row 0 data, row 1 zeros; the
        # rank-1 matmuls use contraction dim 2 with ones on both rows)
        r = np.zeros((2, rw), np.float32)
        r[0, :NTOP] = bias_sh
        for s, dd in enumerate(segs):
            r[0, NTOP + np.arange(K) * nseg + s] = bias_ns[dd]
        r[:, NTOP + nseg * K :] = 1.0

        m = np.zeros((RPC, nseg), np.float32)
        for s, dd in enumerate(segs):
            m[:, s] = (rows == dd).astype(np.float32)

        in_maps.append({
            "a": a.astype(BF16),
            "b": b.astype(BF16),
            "r": r.astype(BF16),
            "m": m,
        })
    return nseg, in_maps, perm


def _finish(results, perm):
    out_sorted = np.concatenate(
        [np.asarray(results[c]["o"], dtype=np.float32) for c in range(NC)], axis=0
    )
    out = np.empty((B, N), np.float32)
    out[perm] = out_sorted
    return out


def get_program(nseg):
    prog = _PROG_CACHE.get(nseg)
    if prog is None:
        prog = _build_program(nseg)
        _PROG_CACHE[nseg] = prog
    return prog


def kernel(epsilon, d, L_emb, S_emb, bias_nonshared, bias_shared):
    from concourse.bass_utils import run_bass_kernel_spmd

    nseg, in_maps, perm = _prepare(
        epsilon, d, L_emb, S_emb, bias_nonshared, bias_shared
    )
    prog = get_program(nseg)
    res = run_bass_kernel_spmd(prog, in_maps, list(range(NC))).results
    return _finish(res, perm)


# revision 7
# speedup vs baseline: 1.2694x; 1.0374x over previous
"""Trainium2 Bass kernel for nn_F_VAE_can_7902739824969.

Reference, per batch row b with domain d = dom[b]:
    out[b] = F_d @ eps[b] + concat(bias_shared, bias_nonshared[d])
with F_d = (I - L_d)^{-1} S_d, L_d strictly-lower only in the last K=64 rows,
S_d diagonal.  Hence F_d = [[I, 0], [F21_d, F22_d]]: only the bottom K rows
(F_bot, [D, K, N]) carry information:
    out[b, :N-K] = eps[b, :N-K] + bias_shared
    out[b, N-K:] = F_bot[d] @ eps[b] + bias_nonshared[d]

Host (inside kernel()): solve the D unit-triangular systems for F_bot, sort
batch rows by domain, give each of 8 cores 128 sorted rows plus the few
domain blocks of F^T that shard touches.  Everything ships as bf16 (the
correctness gate is rel 2e-2; bf16 keeps us ~3e-3).

Device, per core (raw bacc).  All input DMAs are issued in `main`, before
the kernel block, so descriptor generation overlaps block entry; the tiny
bias blob rides the gpsimd SWDGE queue so it never serializes a HWDGE
queue.  PE does ALL the math into two PSUM banks:
    p_top [128,448]: per-chunk identity matmuls (epsT_c^T @ I, the PE
        doubles as the eps transpose engine; I is built on-device with one
        affine_select) + a rank-1 ones x bias_top matmul
    p_bot [128,K*nseg]: per-chunk epsT_c^T @ F^T chunks + rank-1 ones x
        bbot matmul (nonshared bias, segment-interleaved)
DVE casts p_top -> out rows 0:448 early (out-top DMA overlaps the F
matmuls), then selects the right segment of p_bot per batch row with
copy_predicated (segment masks are 0/1 per row = per partition).
Cheap bf16 dummy matmuls keep the PE clock-gate (HAM) open: warm PE both
speeds the real matmuls and speeds the Tensor section of the runtime's
end-of-NEFF semaphore-clear ladder (~118 vs ~140 ns per clear).
"""

import numpy as np
import ml_dtypes

B = 1024
N = 512
K = 64
D = 16
P = 128
NC = 8
RPC = B // NC          # rows per core
NTOP = N - K           # 448
NCHUNK = N // P        # 4 contraction chunks

BF16 = ml_dtypes.bfloat16

# PE keep-warm dummy matmuls (each ~125ns bf16): W_START run while the input
# DMAs are in flight, W_TAIL after the real matmuls while DVE + output DMAs
# finish.  Both only burn otherwise-idle PE time.
W_START = 20
W_TAIL = 10

_PROG_CACHE: dict = {}


def _build_fbot(L_emb, S_emb):
    """F_bot [D, K, N] (float32): bottom K rows of (I - L_d)^{-1} S_d."""
    L_emb = np.asarray(L_emb, np.float64)
    S_emb = np.asarray(S_emb, np.float64)
    off = np.zeros(K, dtype=np.int64)
    for r in range(1, K):
        off[r] = off[r - 1] + (NTOP + r - 1)
    L21 = np.zeros((D, K, NTOP))
    L22 = np.zeros((D, K, K))
    for r in range(K):
        L21[1:, r, :] = L_emb[1:, off[r] : off[r] + NTOP]
        if r > 0:
            L22[1:, r, :r] = L_emb[1:, off[r] + NTOP : off[r] + NTOP + r]
    s = np.ones((D, K))
    s[1:] = S_emb[1:]
    rhs = np.concatenate([L21, s[:, :, None] * np.eye(K)[None]], axis=2)  # [D,K,N]
    X = np.zeros_like(rhs)
    for r in range(K):
        X[:, r, :] = rhs[:, r, :] + np.einsum(
            "dj,djn->dn", L22[:, r, :r], X[:, :r, :]
        )
    return X.astype(np.float32)


def _build_program(nseg):
    import concourse.bacc as bacc
    import concourse.mybir as mybir

    f32 = mybir.dt.float32
    bf16 = mybir.dt.bfloat16

    aw = NCHUNK * P              # epsT chunks
    bw = NCHUNK * nseg * K       # F^T chunks, (k*nseg+s)-interleaved
    rw = NTOP + nseg * K + P     # bias_top | bbot_flat | ones

    nc = bacc.Bacc()
    a_in = nc.declare_dram_parameter("a", [P, aw], bf16, isOutput=False)
    b_in = nc.declare_dram_parameter("b", [P, bw], bf16, isOutput=False)
    r_in = nc.declare_dram_parameter("r", [2, rw], bf16, isOutput=False)
    m_in = nc.declare_dram_parameter("m", [RPC, nseg], mybir.dt.uint8, isOutput=False)
    o_ext = nc.declare_dram_parameter("o", [RPC, N], bf16, isOutput=True)

    a_sb = nc.alloc_sbuf_tensor("a_sb", [P, aw], bf16).ap()
    b_sb = nc.alloc_sbuf_tensor("b_sb", [P, bw], bf16).ap()
    r_sb = nc.alloc_sbuf_tensor("r_sb", [2, rw], bf16).ap()
    m_sb = nc.alloc_sbuf_tensor("m_sb", [P, nseg], mybir.dt.uint8).ap()
    eye = nc.alloc_sbuf_tensor("eye", [P, P], bf16).ap()
    junk = nc.alloc_sbuf_tensor("junk", [P, 32], bf16).ap()
    out_sb = nc.alloc_sbuf_tensor("out_sb", [P, N], bf16).ap()

    p_top = nc.alloc_psum_tensor("p_top", [P, NTOP], f32).ap()
    p_bot = nc.alloc_psum_tensor("p_bot", [P, K, nseg], f32).ap()
    p_scr = nc.alloc_psum_tensor("p_scr", [P, 16], f32).ap()

    ones = r_sb[:, NTOP + nseg * K :]
    bias_top = r_sb[:, :NTOP]
    bbot = r_sb[:, NTOP : NTOP + nseg * K]

    s_junk = nc.alloc_semaphore("s_junk")
    s_a = nc.alloc_semaphore("s_a")
    s_b = nc.alloc_semaphore("s_b")
    s_r = nc.alloc_semaphore("s_r")
    s_m = nc.alloc_semaphore("s_m")
    s_eye = nc.alloc_semaphore("s_eye")
    s_pt = nc.alloc_semaphore("s_pt")
    s_pe = nc.alloc_semaphore("s_pe")
    s_top = nc.alloc_semaphore("s_top")
    s_bot = nc.alloc_semaphore("s_bot")
    s_out = nc.alloc_semaphore("s_out")

    one_bf16 = nc.const_aps.aps[(bf16, 1.0)]

    # ---- main: all input DMAs + on-device constants, before block entry ----
    nc.sync.dma_start(a_sb, a_in[:]).then_inc(s_a, 16)
    nc.scalar.dma_start(b_sb, b_in[:]).then_inc(s_b, 16)
    nc.scalar.dma_start(m_sb, m_in[:]).then_inc(s_m, 16)
    nc.gpsimd.memset(junk, 0).then_inc(s_junk, 1)
    # eye[p, n] = 1.0 where n - p == 0
    nc.gpsimd.affine_select(
        eye,
        one_bf16.to_broadcast([P, P]),
        pattern=[[1, P]],
        compare_op=mybir.AluOpType.is_equal,
        fill=0.0,
        base=0,
        channel_multiplier=-1,
    ).then_inc(s_eye, 1)
    nc.gpsimd.dma_start(r_sb, r_in[:]).then_inc(s_r, 16)  # SWDGE

    with nc.Block() as block:

        @block.tensor
        def _(te):
            te.wait_ge(s_junk, 1)
            for _ in range(W_START):
                te.matmul(p_scr[:16, :16], lhsT=junk[:, :16], rhs=junk[:, 16:],
                          start=True, stop=True)
            # rank-1 bias fill first: p_top = 1 (x) bias_top.  start=True
            # resets has_written for the whole bank, so it must precede the
            # per-chunk writes.
            te.wait_ge(s_r, 16)
            te.matmul(p_top, lhsT=ones, rhs=bias_top, start=True, stop=False)
            # eps transpose: p_top[:, 128c:...] += epsT_c^T @ I
            te.wait_ge(s_a, 16)
            te.wait_ge(s_eye, 1)
            mm = None
            for c in range(NCHUNK):
                w = P if c < NCHUNK - 1 else P - K
                mm = te.matmul(
                    p_top[:, c * P : c * P + w],
                    lhsT=a_sb[:, c * P : (c + 1) * P],
                    rhs=eye[:, :w],
                    start=False, stop=True,
                )
            mm.then_inc(s_pt, 1)
            # bottom: p_bot = sum_c epsT_c^T @ F^T_c + 1 (x) bbot
            te.wait_ge(s_b, 16)
            for c in range(NCHUNK):
                te.matmul(
                    p_bot,
                    lhsT=a_sb[:, c * P : (c + 1) * P],
                    rhs=b_sb[:, c * nseg * K : (c + 1) * nseg * K],
                    start=(c == 0), stop=False,
                )
            mm = te.matmul(p_bot.rearrange("p k s -> p (k s)"),
                           lhsT=ones, rhs=bbot, start=False, stop=True)
            mm.then_inc(s_pe, 1)
            for _ in range(W_TAIL):
                te.matmul(p_scr[:16, :16], lhsT=junk[:, :16], rhs=junk[:, 16:],
                          start=True, stop=True)

        @block.sync
        def _(sy):
            sy.wait_ge(s_top, 1)
            sy.dma_start(o_ext[:, :NTOP], out_sb[:, :NTOP]).then_inc(s_out, 16)

        @block.scalar
        def _(sc):
            sc.wait_ge(s_bot, 1)
            sc.dma_start(o_ext[:, NTOP:], out_sb[:, NTOP:]).then_inc(s_out, 16)

        @block.vector
        def _(ve):
            ve.wait_ge(s_pt, 1)
            ve.tensor_copy(out_sb[:, :NTOP], p_top).then_inc(s_top, 1)
            ve.wait_ge(s_pe, 1)
            ve.wait_ge(s_m, 16)
            mm = None
            for s in range(nseg):
                mm = ve.copy_predicated(
                    out_sb[:, NTOP:],
                    m_sb[:, s, None].to_broadcast([P, K]),
                    p_bot[:, :, s],
                )
            mm.then_inc(s_bot, 1)

    nc.compile()
    return nc


def _prepare(epsilon, d, L_emb, S_emb, bias_nonshared, bias_shared):
    """Host-side sharding. Returns (nseg, in_maps, perm)."""
    eps = np.ascontiguousarray(np.asarray(epsilon, np.float32))
    dv = np.asarray(d).astype(np.int64).reshape(B)
    bias_ns = np.asarray(bias_nonshared, np.float32)
    bias_sh = np.asarray(bias_shared, np.float32).reshape(NTOP)

    fbot = _build_fbot(L_emb, S_emb)                     # [D, K, N]

    perm = np.argsort(dv, kind="stable")
    ds_sorted = dv[perm]
    eps_sorted = eps[perm]

    shard_segs = []
    for c in range(NC):
        rows = ds_sorted[c * RPC : (c + 1) * RPC]
        segs = []
        for dd in rows:
            if not segs or segs[-1] != dd:
                segs.append(int(dd))
        shard_segs.append(segs)
    nseg = max(len(s) for s in shard_segs)

    rw = NTOP + nseg * K + P
    in_maps = []
    for c in range(NC):
        segs = shard_segs[c]
        rows = ds_sorted[c * RPC : (c + 1) * RPC]
        eps_c = eps_sorted[c * RPC : (c + 1) * RPC]          # [128, 512]

        # a: epsT chunks.  a[p, cc*128 + r] = eps[r, cc*128 + p]
        est = eps_c.T.reshape(NCHUNK, P, RPC)                # [cc, p, r]
        a = np.ascontiguousarray(
            est.transpose(1, 0, 2).reshape(P, NCHUNK * P)
        )

        # b: F^T chunks, col (cc, k, s) -> fbot[dom_s, k, cc*128+p]
        b = np.zeros((P, NCHUNK, K, nseg), np.float32)
        for s, dd in enumerate(segs):
            b[:, :, :, s] = fbot[dd].T.reshape(NCHUNK, P, K).transpose(1, 0, 2)
        b = b.reshape(P, NCHUNK * nseg * K)

        # r: bias_top | bbot_flat | ones (row 0 data, row 1 zeros; the
        # rank-1 matmuls use contraction dim 2 with ones on both rows)
        r = np.zeros((2, rw), np.float32)
        r[0, :NTOP] = bias_sh
        for s, dd in enumerate(segs):
            r[0, NTOP + np.arange(K) * nseg + s] = bias_ns[dd]
        r[:, NTOP + nseg * K :] = 1.0

        m = np.zeros((RPC, nseg), np.uint8)
        for s, dd in enumerate(segs):
            m[:, s] = (rows == dd).astype(np.uint8)

        in_maps.append({
            "a": a.astype(BF16),
            "b": b.astype(BF16),
            "r": r.astype(BF16),
            "m": m,
        })
    return nseg, in_maps, perm


def _finish(results, perm):
    out_sorted = np.concatenate(
        [np.asarray(results[c]["o"], dtype=np.float32) for c in range(NC)], axis=0
    )
    out = np.empty((B, N), np.float32)
    out[perm] = out_sorted
    return out


def get_program(nseg):
    prog = _PROG_CACHE.get(nseg)
    if prog is None:
        prog = _build_program(nseg)
        _PROG_CACHE[nseg] = prog
    return prog


def kernel(epsilon, d, L_emb, S_emb, bias_nonshared, bias_shared):
    from concourse.bass_utils import run_bass_kernel_spmd

    nseg, in_maps, perm = _prepare(
        epsilon, d, L_emb, S_emb, bias_nonshared, bias_shared
    )
    prog = get_program(nseg)
    res = run_bass_kernel_spmd(prog, in_maps, list(range(NC))).results
    return _finish(res, perm)
